# revision 21
# baseline (speedup 1.0000x reference)
"""Trainium2 Bass kernel for nn_Encoder_79585743995180 (sparse_attention).

Self-contained: hardcodes shapes/sharding. Strategy (validated in numpy):
  - 8 cores, head-parallel: core c owns heads {2c, 2c+1} (128 of 1024 dims).
  - Per core: q/k/v projections for its 128 dims (reads full activations,
    sliced weights), rope (de-interleaved even/odd permutation so the
    rotation partner sits at partition offset +32 within each 64-dim head
    block), main attention with column-softmax folded into a 1/colsum
    prescale of the AV stationary operand, memory attention with mask+gate
    folded into the host-prepped vmaug tensor, out_proj partial product.
  - Host sums the 8 partial outputs (contraction-sharded out_proj).
  - Matmul operands in bf16 (fp32 matmuls are split into hi/lo passes on
    trn2 PE = 2x instructions); accumulation stays fp32 in PSUM, and the
    softmax renormalization path stays fp32.

All biases in this problem are zeros (spec fill=zeros) and are skipped.
The reference's `+1e-8` softmax epsilon is omitted (validated: rel err
~4e-6 vs reference in fp32).

Layout conventions on device (per core):
  qT/kT   (128 dims, 4096 rows) bf16   rows r = n*L + l, dims rope-permuted
  v       rows layout, stored as v_sb (128 rows%128, 32 rowtile, 2 head, 65)
          bf16, with ones in column 64 (renorm denominator rides the AV mm)
  attnT   (128 dims, 4096 rows) bf16
  outT    (1024, 4096) fp32 partial, host sums across cores.
"""

import ml_dtypes
import numpy as np

import concourse.bass as bass
import concourse.bacc as bacc
import concourse.mybir as mybir
import concourse.tile as tile
from concourse import bass_utils

F32 = mybir.dt.float32
BF16 = mybir.dt.float16
NPBF = np.float16
AF = mybir.ActivationFunctionType

L = 1024
S = 1024
N = 4
E = 1024
H = 16
D = 64
M = 512
NC = 8
HPC = H // NC          # 2 heads per core
DC = HPC * D           # 128 dims per core
R = L * N              # 4096 rows, r = n*L + l

_COMPILED = {}


def _build(dbg=False):
    nc = bacc.Bacc("TRN2", target_bir_lowering=False, debug=False)

    # ---- DRAM I/O ----
    xqT = nc.dram_tensor("xqT", [E, R], BF16, kind="ExternalInput").ap()
    xkT = nc.dram_tensor("xkT", [E, R], BF16, kind="ExternalInput").ap()
    xvT = nc.dram_tensor("xvT", [E, R], BF16, kind="ExternalInput").ap()
    wqT = nc.dram_tensor("wqT", [E, DC], BF16, kind="ExternalInput").ap()
    wkT = nc.dram_tensor("wkT", [E, DC], BF16, kind="ExternalInput").ap()
    wvT = nc.dram_tensor("wvT", [E, DC], BF16, kind="ExternalInput").ap()
    woT = nc.dram_tensor("woT", [DC, E], BF16, kind="ExternalInput").ap()
    cosq = nc.dram_tensor("cosq", [DC, R], BF16, kind="ExternalInput").ap()
    sinq = nc.dram_tensor("sinq", [DC, R], BF16, kind="ExternalInput").ap()
    cosk = nc.dram_tensor("cosk", [DC, R], BF16, kind="ExternalInput").ap()
    sink = nc.dram_tensor("sink", [DC, R], BF16, kind="ExternalInput").ap()
    kmem = nc.dram_tensor("kmem", [DC, N, M], BF16, kind="ExternalInput").ap()
    vmaug = nc.dram_tensor("vmaug", [128, N, HPC, 4, 65], BF16,
                           kind="ExternalInput").ap()
    outT = nc.dram_tensor("outT", [E, R], F32, kind="ExternalOutput").ap()
    dbg_t = {}
    if dbg:
        for nm, shp in (("dbg_q", [DC, R]), ("dbg_k", [DC, R]),
                        ("dbg_attn", [DC, R])):
            dbg_t[nm] = nc.dram_tensor(nm, shp, F32, kind="ExternalOutput").ap()

    with tile.TileContext(nc) as tc:
        with (
            tc.tile_pool(name="const", bufs=1) as const,
            tc.tile_pool(name="persist", bufs=1) as persist,
            tc.tile_pool(name="xstream", bufs=4) as xstream,
            tc.tile_pool(name="cs", bufs=2) as cs,
            tc.tile_pool(name="scratch", bufs=3) as scratch,
            tc.tile_pool(name="attnscr", bufs=2) as attnscr,
            tc.tile_pool(name="rows", bufs=1) as rows,
            tc.tile_pool(name="drows", bufs=2, space="DRAM") as drows,
            tc.tile_pool(name="wexp", bufs=10) as wexpp,
            tc.tile_pool(name="small", bufs=3) as small,
            tc.tile_pool(name="ostage", bufs=4) as ostage,
            tc.tile_pool(name="pw", bufs=2, space="PSUM") as pw,
            tc.tile_pool(name="pproj", bufs=2, space="PSUM") as pproj,
            tc.tile_pool(name="pacc", bufs=1, space="PSUM") as pacc,
        ):
            # ---- constants into SBUF ----
            w_sb = {}
            for name, src in (("q", wqT), ("k", wkT), ("v", wvT)):
                t = const.tile([128, 8, DC], BF16, tag=f"w_{name}")
                nc.sync.dma_start(
                    out=t, in_=src.rearrange("(kc p) d -> p kc d", p=128))
                w_sb[name] = t
            wo_sb = const.tile([DC, E], BF16)
            nc.sync.dma_start(out=wo_sb, in_=woT)
            kmem_sb = const.tile([DC, N, M], BF16)
            nc.sync.dma_start(out=kmem_sb, in_=kmem)
            vmaug_sb = const.tile([128, N, HPC, 4, 65], BF16)
            nc.sync.dma_start(out=vmaug_sb, in_=vmaug)

            # per-n persistent tiles so Tile can pipeline proj(n+1)
            # under attn/outproj(n)
            qT_n = [persist.tile([DC, L], BF16, tag=f"qT{n}", name=f"qT{n}") for n in range(N)]
            kT_n = [persist.tile([DC, L], BF16, tag=f"kT{n}", name=f"kT{n}") for n in range(N)]
            v_n = [persist.tile([128, 8, HPC, 65], BF16, tag=f"v{n}",
                                name=f"v{n}") for n in range(N)]
            attn_n = [persist.tile([DC, L], BF16, tag=f"at{n}",
                                   name=f"at{n}") for n in range(N)]
            for n in range(N):
                nc.vector.memset(v_n[n][:, :, :, 64:65], 1.0)

            for n in range(N):
                # ---- projections for batch n (rows n*L .. n*L+L) ----
                for rt2 in range(2):
                    rt = n * 2 + rt2
                    rs = slice(rt * 512, (rt + 1) * 512)
                    ls = slice(rt2 * 512, (rt2 + 1) * 512)
                    for name, xT, cosT, sinT in (
                        ("q", xqT, cosq, sinq),
                        ("k", xkT, cosk, sink),
                    ):
                        dest = qT_n[n] if name == "q" else kT_n[n]
                        xs = xstream.tile([128, 8, 512], BF16, tag="xs")
                        nc.sync.dma_start(
                            out=xs,
                            in_=xT[:, rs].rearrange("(kc p) r -> p kc r", p=128))
                        ps = pproj.tile([128, 512], F32, tag="pp")
                        for kc in range(8):
                            nc.tensor.matmul(
                                ps, w_sb[name][:, kc, :], xs[:, kc, :],
                                start=(kc == 0), stop=(kc == 7))
                        raw = scratch.tile([128, 512], BF16, tag="raw")
                        nc.scalar.activation(raw, ps, AF.Copy)
                        sw = scratch.tile([128, 512], BF16, tag="sw")
                        for hb in range(HPC):
                            b = hb * 64
                            nc.scalar.dma_start(
                                out=sw[b:b + 32, :], in_=raw[b + 32:b + 64, :])
                            nc.scalar.dma_start(
                                out=sw[b + 32:b + 64, :], in_=raw[b:b + 32, :])
                        ct = cs.tile([128, 512], BF16, tag="ct")
                        st = cs.tile([128, 512], BF16, tag="st")
                        nc.sync.dma_start(out=ct, in_=cosT[:, rs])
                        nc.sync.dma_start(out=st, in_=sinT[:, rs])
                        t1 = scratch.tile([128, 512], BF16, tag="t1")
                        nc.vector.tensor_mul(t1, raw, ct)
                        t2 = scratch.tile([128, 512], BF16, tag="t2")
                        nc.vector.tensor_mul(t2, sw, st)
                        nc.vector.tensor_add(dest[:, ls], t1, t2)
                    # v projection for these rows
                    xs = xstream.tile([128, 8, 512], BF16, tag="xs")
                    nc.sync.dma_start(
                        out=xs,
                        in_=xvT[:, rs].rearrange("(kc p) r -> p kc r", p=128))
                    for st_i in range(4):
                        ps = pproj.tile([128, 512], F32, tag="pp")
                        for kc in range(8):
                            nc.tensor.matmul(
                                ps[:, 0:128],
                                xs[:, kc, st_i * 128:(st_i + 1) * 128],
                                w_sb["v"][:, kc, :],
                                start=(kc == 0), stop=(kc == 7))
                        t = rt2 * 4 + st_i
                        for h in range(HPC):
                            nc.scalar.activation(
                                v_n[n][:, t, h, 0:64],
                                ps[:, h * 64:(h + 1) * 64], AF.Copy)

                # ---- attention for batch n, both heads ----
                for h in range(HPC):
                    ho = h * 64
                    colsum = small.tile([128, 8], F32, tag="colsum")
                    wxs = []
                    for sc in range(8):
                        pwt = pw.tile([128, 1024], F32, tag="pw")
                        for lc in range(2):
                            nc.tensor.matmul(
                                pwt[:, lc * 512:(lc + 1) * 512],
                                kT_n[n][ho:ho + 64,
                                        sc * 128:(sc + 1) * 128],
                                qT_n[n][ho:ho + 64,
                                        lc * 512:(lc + 1) * 512],
                                start=True, stop=True)
                        wx = wexpp.tile([128, 1024], BF16, tag="wx")
                        nc.scalar.activation(
                            wx, pwt, AF.Exp, accum_out=colsum[:, sc:sc + 1])
                        wxs.append(wx)
                    rcall = small.tile([128, 8], F32, tag="rcall")
                    nc.vector.reciprocal_approx_fast(rcall, colsum)
                    pmain = pacc.tile([65, 1024], F32, tag="pmain")
                    for sc in range(8):
                        vs = small.tile([128, 65], BF16, tag="vs")
                        nc.vector.tensor_scalar_mul(
                            vs, v_n[n][:, sc, h, :], rcall[:, sc:sc + 1])
                        for lc in range(2):
                            nc.tensor.matmul(
                                pmain[:, lc * 512:(lc + 1) * 512],
                                vs, wxs[sc][:, lc * 512:(lc + 1) * 512],
                                start=(sc == 0), stop=(sc == 7))
                    pmem = pacc.tile([65, 1024], F32, tag="pmain")
                    for mc in range(4):
                        pwt = pw.tile([128, 1024], F32, tag="pw")
                        for lc in range(2):
                            nc.tensor.matmul(
                                pwt[:, lc * 512:(lc + 1) * 512],
                                kmem_sb[ho:ho + 64, n,
                                        mc * 128:(mc + 1) * 128],
                                qT_n[n][ho:ho + 64,
                                        lc * 512:(lc + 1) * 512],
                                start=True, stop=True)
                        wx = wexpp.tile([128, 1024], BF16, tag="wx")
                        nc.scalar.activation(wx, pwt, AF.Exp)
                        for lc in range(2):
                            nc.tensor.matmul(
                                pmem[:, lc * 512:(lc + 1) * 512],
                                vmaug_sb[:, n, h, mc, :],
                                wx[:, lc * 512:(lc + 1) * 512],
                                start=(mc == 0), stop=(mc == 3))
                    smain = attnscr.tile([65, 1024], F32, tag="smain")
                    smem = attnscr.tile([65, 1024], F32, tag="smem")
                    nc.scalar.activation(smain, pmain, AF.Copy)
                    nc.vector.tensor_copy(smem, pmem)
                    d1 = rows.tile([1, 1024], F32, tag="d1")
                    d2 = rows.tile([1, 1024], F32, tag="d2")
                    nc.scalar.dma_start(out=d1, in_=smain[64:65, :])
                    nc.scalar.dma_start(out=d2, in_=smem[64:65, :])
                    r1 = rows.tile([1, 1024], F32, tag="r1")
                    r2 = rows.tile([1, 1024], F32, tag="r2")
                    nc.vector.reciprocal_approx_fast(r1, d1)
                    nc.vector.reciprocal_approx_fast(r2, d2)
                    dr1 = drows.tile([1, 1024], F32, tag="dr1")
                    dr2 = drows.tile([1, 1024], F32, tag="dr2")
                    nc.scalar.dma_start(out=dr1, in_=r1)
                    nc.scalar.dma_start(out=dr2, in_=r2)
                    bc1 = attnscr.tile([64, 1024], F32, tag="bc1")
                    bc2 = attnscr.tile([64, 1024], F32, tag="bc2")
                    nc.scalar.dma_start(
                        out=bc1, in_=dr1.to_broadcast((64, 1024)))
                    nc.scalar.dma_start(
                        out=bc2, in_=dr2.to_broadcast((64, 1024)))
                    u1 = attnscr.tile([64, 1024], F32, tag="u1")
                    nc.vector.tensor_mul(u1, smain[0:64, :], bc1)
                    u2 = attnscr.tile([64, 1024], F32, tag="u2")
                    nc.vector.tensor_mul(u2, smem[0:64, :], bc2)
                    nc.vector.tensor_add(attn_n[n][ho:ho + 64, :], u1, u2)

                if dbg:
                    nc.sync.dma_start(
                        out=dbg_t["dbg_q"][:, n * L:(n + 1) * L], in_=qT_n[n])
                    nc.sync.dma_start(
                        out=dbg_t["dbg_k"][:, n * L:(n + 1) * L], in_=kT_n[n])
                    nc.sync.dma_start(
                        out=dbg_t["dbg_attn"][:, n * L:(n + 1) * L],
                        in_=attn_n[n])

                # ---- out_proj partial for batch n ----
                for oc in range(8):
                    for rt2 in range(2):
                        po = pproj.tile([128, 512], F32, tag="pp")
                        nc.tensor.matmul(
                            po, wo_sb[:, oc * 128:(oc + 1) * 128],
                            attn_n[n][:, rt2 * 512:(rt2 + 1) * 512],
                            start=True, stop=True)
                        so = ostage.tile([128, 512], F32, tag="so")
                        dst = outT[oc * 128:(oc + 1) * 128,
                                   n * L + rt2 * 512:n * L + (rt2 + 1) * 512]
                        if (oc + rt2) % 2 == 0:
                            nc.scalar.activation(so, po, AF.Copy)
                            nc.scalar.dma_start(out=dst, in_=so)
                        else:
                            nc.vector.tensor_copy(so, po)
                            nc.gpsimd.dma_start(out=dst, in_=so)

    nc.compile()
    return nc


def _perm64():
    p = np.empty(64, np.int64)
    p[:32] = np.arange(0, 64, 2)
    p[32:] = np.arange(1, 64, 2)
    return p


def _prep_inputs(inputs):
    """Host-side shard prep. Returns list of per-core input dicts."""
    f = np.float32
    query = np.asarray(inputs["query"], f)
    key = np.asarray(inputs["key"], f)
    value = np.asarray(inputs["value"], f)
    W = np.asarray(inputs["in_proj_weight"], f)
    wo = np.asarray(inputs["out_proj_weight"], f)
    qp = np.asarray(inputs["qp"], f)
    kvp = np.asarray(inputs["kvp"], f)
    k_mem = np.asarray(inputs["k_mem"], f)
    v_mem = np.asarray(inputs["v_mem"], f)
    gate = np.asarray(inputs["gate_attn"], f)
    mask = np.asarray(inputs["mem_mask"]).astype(f)

    g = 1.0 / (1.0 + np.exp(-gate))
    perm64 = _perm64()
    sgn = np.concatenate([np.full(32, -1.0, f), np.full(32, 1.0, f)] * HPC)

    xqT = np.ascontiguousarray(
        query.transpose(2, 1, 0).reshape(E, R)).astype(NPBF)
    xkT = np.ascontiguousarray(
        key.transpose(2, 1, 0).reshape(E, R)).astype(NPBF)
    xvT = np.ascontiguousarray(
        value.transpose(2, 1, 0).reshape(E, R)).astype(NPBF)

    in_maps = []
    for c in range(NC):
        dims = np.arange(c * DC, (c + 1) * DC)
        dims_perm = np.concatenate([dims[h * 64 + perm64] for h in range(HPC)])
        gv = np.concatenate(
            [np.full(64, 1.0 - g[2 * c + h], f) for h in range(HPC)])

        wq = W[:E][dims_perm] * np.float32(D ** -0.5)
        wk = W[E:2 * E][dims_perm]
        wv = W[2 * E:][dims] * gv[:, None]

        def rope(pe):
            cosT = np.ascontiguousarray(
                pe[:, :, dims_perm, 0].transpose(2, 0, 1).reshape(DC, R))
            sinT = np.ascontiguousarray(
                pe[:, :, dims_perm, 1].transpose(2, 0, 1).reshape(DC, R)
                * sgn[:, None])
            return cosT.astype(NPBF), sinT.astype(NPBF)

        cq, sq = rope(qp)
        ck, sk = rope(kvp)

        kmemT = np.ascontiguousarray(
            k_mem[:, dims_perm, :].transpose(1, 0, 2)).astype(NPBF)

        vma = np.zeros((N, HPC, M, 65), f)
        for n in range(N):
            for h in range(HPC):
                gh = g[2 * c + h]
                vm = v_mem[n, dims[h * 64:(h + 1) * 64], :].T  # (M, 64)
                vma[n, h, :, :64] = vm * gh * mask[n][:, None]
                vma[n, h, :, 64] = mask[n]
        vma_dev = np.ascontiguousarray(
            vma.reshape(N, HPC, 4, 128, 65).transpose(3, 0, 1, 2, 4)).astype(NPBF)

        in_maps.append({
            "xqT": xqT, "xkT": xkT, "xvT": xvT,
            "wqT": np.ascontiguousarray(wq.T).astype(NPBF),
            "wkT": np.ascontiguousarray(wk.T).astype(NPBF),
            "wvT": np.ascontiguousarray(wv.T).astype(NPBF),
            "woT": np.ascontiguousarray(wo[:, dims].T).astype(NPBF),
            "cosq": cq, "sinq": sq, "cosk": ck, "sink": sk,
            "kmem": kmemT, "vmaug": vma_dev,
        })
    return in_maps


def kernel(**inputs):
    if "nc" not in _COMPILED:
        _COMPILED["nc"] = _build()
    nc = _COMPILED["nc"]
    in_maps = _prep_inputs(inputs)
    res = bass_utils.run_bass_kernel_spmd(nc, in_maps, core_ids=list(range(NC)))
    total = np.zeros((E, R), np.float64)
    for r in res.results:
        total += r["outT"].astype(np.float64)
    out = total.T.reshape(N, L, E).transpose(1, 0, 2).astype(np.float32)
    out = out + np.asarray(inputs["out_proj_bias"], np.float32)
    return out


# revision 23
# speedup vs baseline: 1.0477x; 1.0477x over previous
"""Trainium2 Bass kernel for nn_Encoder_79585743995180 (sparse_attention).

Self-contained: hardcodes shapes/sharding. Strategy (validated in numpy):
  - 8 cores, head-parallel: core c owns heads {2c, 2c+1} (128 of 1024 dims).
  - Per core: q/k/v projections for its 128 dims (reads full activations,
    sliced weights), rope (de-interleaved even/odd permutation so the
    rotation partner sits at partition offset +32 within each 64-dim head
    block), main attention with column-softmax folded into a 1/colsum
    prescale of the AV stationary operand, memory attention with mask+gate
    folded into the host-prepped vmaug tensor, out_proj partial product.
  - Host sums the 8 partial outputs (contraction-sharded out_proj).
  - Matmul operands in bf16 (fp32 matmuls are split into hi/lo passes on
    trn2 PE = 2x instructions); accumulation stays fp32 in PSUM, and the
    softmax renormalization path stays fp32.

All biases in this problem are zeros (spec fill=zeros) and are skipped.
The reference's `+1e-8` softmax epsilon is omitted (validated: rel err
~4e-6 vs reference in fp32).

Layout conventions on device (per core):
  qT/kT   (128 dims, 4096 rows) bf16   rows r = n*L + l, dims rope-permuted
  v       rows layout, stored as v_sb (128 rows%128, 32 rowtile, 2 head, 65)
          bf16, with ones in column 64 (renorm denominator rides the AV mm)
  attnT   (128 dims, 4096 rows) bf16
  outT    (1024, 4096) fp32 partial, host sums across cores.
"""

import ml_dtypes
import numpy as np

import concourse.bass as bass
import concourse.bacc as bacc
import concourse.mybir as mybir
import concourse.tile as tile
from concourse import bass_utils

F32 = mybir.dt.float32
BF16 = mybir.dt.float16
NPBF = np.float16
AF = mybir.ActivationFunctionType

L = 1024
S = 1024
N = 4
E = 1024
H = 16
D = 64
M = 512
NC = 8
HPC = H // NC          # 2 heads per core
DC = HPC * D           # 128 dims per core
R = L * N              # 4096 rows, r = n*L + l

_COMPILED = {}


def _build(dbg=False):
    nc = bacc.Bacc("TRN2", target_bir_lowering=False, debug=False)

    # ---- DRAM I/O ----
    xqT = nc.dram_tensor("xqT", [E, R], BF16, kind="ExternalInput").ap()
    xkT = nc.dram_tensor("xkT", [E, R], BF16, kind="ExternalInput").ap()
    xvT = nc.dram_tensor("xvT", [E, R], BF16, kind="ExternalInput").ap()
    wqT = nc.dram_tensor("wqT", [E, DC], BF16, kind="ExternalInput").ap()
    wkT = nc.dram_tensor("wkT", [E, DC], BF16, kind="ExternalInput").ap()
    wvT = nc.dram_tensor("wvT", [E, DC], BF16, kind="ExternalInput").ap()
    woT = nc.dram_tensor("woT", [DC, E], BF16, kind="ExternalInput").ap()
    cosq = nc.dram_tensor("cosq", [DC, R], BF16, kind="ExternalInput").ap()
    sinq = nc.dram_tensor("sinq", [DC, R], BF16, kind="ExternalInput").ap()
    cosk = nc.dram_tensor("cosk", [DC, R], BF16, kind="ExternalInput").ap()
    sink = nc.dram_tensor("sink", [DC, R], BF16, kind="ExternalInput").ap()
    kmem = nc.dram_tensor("kmem", [DC, N, M], BF16, kind="ExternalInput").ap()
    vmaug = nc.dram_tensor("vmaug", [128, N, HPC, 4, 65], BF16,
                           kind="ExternalInput").ap()
    outT = nc.dram_tensor("outT", [E, R], F32, kind="ExternalOutput").ap()
    dbg_t = {}
    if dbg:
        for nm, shp in (("dbg_q", [DC, R]), ("dbg_k", [DC, R]),
                        ("dbg_attn", [DC, R])):
            dbg_t[nm] = nc.dram_tensor(nm, shp, F32, kind="ExternalOutput").ap()

    with tile.TileContext(nc) as tc:
        with (
            tc.tile_pool(name="const", bufs=1) as const,
            tc.tile_pool(name="persist", bufs=1) as persist,
            tc.tile_pool(name="xstream", bufs=4) as xstream,
            tc.tile_pool(name="cs", bufs=2) as cs,
            tc.tile_pool(name="scratch", bufs=3) as scratch,
            tc.tile_pool(name="attnscr", bufs=2) as attnscr,
            tc.tile_pool(name="rows", bufs=1) as rows,
            tc.tile_pool(name="drows", bufs=2, space="DRAM") as drows,
            tc.tile_pool(name="wexp", bufs=10) as wexpp,
            tc.tile_pool(name="small", bufs=3) as small,
            tc.tile_pool(name="ostage", bufs=4) as ostage,
            tc.tile_pool(name="pw", bufs=2, space="PSUM") as pw,
            tc.tile_pool(name="pproj", bufs=2, space="PSUM") as pproj,
            tc.tile_pool(name="pacc", bufs=1, space="PSUM") as pacc,
        ):
            # ---- constants into SBUF ----
            w_sb = {}
            for name, src in (("q", wqT), ("k", wkT), ("v", wvT)):
                t = const.tile([128, 8, DC], BF16, tag=f"w_{name}")
                nc.sync.dma_start(
                    out=t, in_=src.rearrange("(kc p) d -> p kc d", p=128))
                w_sb[name] = t
            wo_sb = const.tile([DC, E], BF16)
            nc.sync.dma_start(out=wo_sb, in_=woT)
            kmem_sb = const.tile([DC, N, M], BF16)
            nc.sync.dma_start(out=kmem_sb, in_=kmem)
            vmaug_sb = const.tile([128, N, HPC, 4, 65], BF16)
            nc.sync.dma_start(out=vmaug_sb, in_=vmaug)

            # per-n persistent tiles so Tile can pipeline proj(n+1)
            # under attn/outproj(n)
            qT_n = [persist.tile([DC, L], BF16, tag=f"qT{n}", name=f"qT{n}") for n in range(N)]
            kT_n = [persist.tile([DC, L], BF16, tag=f"kT{n}", name=f"kT{n}") for n in range(N)]
            v_n = [persist.tile([128, 8, HPC, 65], BF16, tag=f"v{n}",
                                name=f"v{n}") for n in range(N)]
            attn_n = [persist.tile([DC, L], BF16, tag=f"at{n}",
                                   name=f"at{n}") for n in range(N)]
            for n in range(N):
                nc.vector.memset(v_n[n][:, :, :, 64:65], 1.0)

            for n in range(N):
                # ---- projections for batch n (rows n*L .. n*L+L) ----
                for rt2 in range(2):
                    rt = n * 2 + rt2
                    rs = slice(rt * 512, (rt + 1) * 512)
                    ls = slice(rt2 * 512, (rt2 + 1) * 512)
                    for name, xT, cosT, sinT in (
                        ("q", xqT, cosq, sinq),
                        ("k", xkT, cosk, sink),
                    ):
                        dest = qT_n[n] if name == "q" else kT_n[n]
                        xs = xstream.tile([128, 8, 512], BF16, tag="xs")
                        nc.sync.dma_start(
                            out=xs,
                            in_=xT[:, rs].rearrange("(kc p) r -> p kc r", p=128))
                        ps = pproj.tile([128, 512], F32, tag="pp")
                        for kc in range(8):
                            nc.tensor.matmul(
                                ps, w_sb[name][:, kc, :], xs[:, kc, :],
                                start=(kc == 0), stop=(kc == 7))
                        ct = cs.tile([128, 512], BF16, tag="ct")
                        st = cs.tile([128, 512], BF16, tag="st")
                        nc.sync.dma_start(out=ct, in_=cosT[:, rs])
                        nc.sync.dma_start(out=st, in_=sinT[:, rs])
                        t1 = scratch.tile([128, 512], BF16, tag="t1")
                        nc.vector.tensor_mul(t1, ps, ct)
                        z = scratch.tile([128, 512], BF16, tag="z")
                        nc.vector.tensor_mul(z, ps, st)
                        t2 = scratch.tile([128, 512], BF16, tag="t2")
                        for hb in range(HPC):
                            b = hb * 64
                            nc.gpsimd.dma_start(
                                out=t2[b:b + 32, :], in_=z[b + 32:b + 64, :])
                            nc.gpsimd.dma_start(
                                out=t2[b + 32:b + 64, :], in_=z[b:b + 32, :])
                        nc.vector.tensor_add(dest[:, ls], t1, t2)
                    # v projection for these rows
                    xs = xstream.tile([128, 8, 512], BF16, tag="xs")
                    nc.sync.dma_start(
                        out=xs,
                        in_=xvT[:, rs].rearrange("(kc p) r -> p kc r", p=128))
                    for st_i in range(4):
                        ps = pproj.tile([128, 512], F32, tag="pp")
                        for kc in range(8):
                            nc.tensor.matmul(
                                ps[:, 0:128],
                                xs[:, kc, st_i * 128:(st_i + 1) * 128],
                                w_sb["v"][:, kc, :],
                                start=(kc == 0), stop=(kc == 7))
                        t = rt2 * 4 + st_i
                        for h in range(HPC):
                            nc.scalar.activation(
                                v_n[n][:, t, h, 0:64],
                                ps[:, h * 64:(h + 1) * 64], AF.Copy)

                # ---- attention for batch n, both heads ----
                for h in range(HPC):
                    ho = h * 64
                    colsum = small.tile([128, 8], F32, tag="colsum")
                    wxs = []
                    for sc in range(8):
                        pwt = pw.tile([128, 1024], F32, tag="pw")
                        for lc in range(2):
                            nc.tensor.matmul(
                                pwt[:, lc * 512:(lc + 1) * 512],
                                kT_n[n][ho:ho + 64,
                                        sc * 128:(sc + 1) * 128],
                                qT_n[n][ho:ho + 64,
                                        lc * 512:(lc + 1) * 512],
                                start=True, stop=True)
                        wx = wexpp.tile([128, 1024], BF16, tag="wx")
                        nc.scalar.activation(
                            wx, pwt, AF.Exp, accum_out=colsum[:, sc:sc + 1])
                        wxs.append(wx)
                    rcall = small.tile([128, 8], F32, tag="rcall")
                    nc.vector.reciprocal_approx_fast(rcall, colsum)
                    pmain = pacc.tile([65, 1024], F32, tag="pmain")
                    for sc in range(8):
                        vs = small.tile([128, 65], BF16, tag="vs")
                        nc.vector.tensor_scalar_mul(
                            vs, v_n[n][:, sc, h, :], rcall[:, sc:sc + 1])
                        for lc in range(2):
                            nc.tensor.matmul(
                                pmain[:, lc * 512:(lc + 1) * 512],
                                vs, wxs[sc][:, lc * 512:(lc + 1) * 512],
                                start=(sc == 0), stop=(sc == 7))
                    pmem = pacc.tile([65, 1024], F32, tag="pmain")
                    for mc in range(4):
                        pwt = pw.tile([128, 1024], F32, tag="pw")
                        for lc in range(2):
                            nc.tensor.matmul(
                                pwt[:, lc * 512:(lc + 1) * 512],
                                kmem_sb[ho:ho + 64, n,
                                        mc * 128:(mc + 1) * 128],
                                qT_n[n][ho:ho + 64,
                                        lc * 512:(lc + 1) * 512],
                                start=True, stop=True)
                        wx = wexpp.tile([128, 1024], BF16, tag="wx")
                        nc.scalar.activation(wx, pwt, AF.Exp)
                        for lc in range(2):
                            nc.tensor.matmul(
                                pmem[:, lc * 512:(lc + 1) * 512],
                                vmaug_sb[:, n, h, mc, :],
                                wx[:, lc * 512:(lc + 1) * 512],
                                start=(mc == 0), stop=(mc == 3))
                    smain = attnscr.tile([65, 1024], F32, tag="smain")
                    smem = attnscr.tile([65, 1024], F32, tag="smem")
                    nc.scalar.activation(smain, pmain, AF.Copy)
                    nc.vector.tensor_copy(smem, pmem)
                    d1 = rows.tile([1, 1024], F32, tag="d1")
                    d2 = rows.tile([1, 1024], F32, tag="d2")
                    nc.gpsimd.dma_start(out=d1, in_=smain[64:65, :])
                    nc.gpsimd.dma_start(out=d2, in_=smem[64:65, :])
                    r1 = rows.tile([1, 1024], F32, tag="r1")
                    r2 = rows.tile([1, 1024], F32, tag="r2")
                    nc.vector.reciprocal_approx_fast(r1, d1)
                    nc.vector.reciprocal_approx_fast(r2, d2)
                    dr1 = drows.tile([1, 1024], F32, tag="dr1")
                    dr2 = drows.tile([1, 1024], F32, tag="dr2")
                    nc.gpsimd.dma_start(out=dr1, in_=r1)
                    nc.gpsimd.dma_start(out=dr2, in_=r2)
                    bc1 = attnscr.tile([64, 1024], F32, tag="bc1")
                    bc2 = attnscr.tile([64, 1024], F32, tag="bc2")
                    nc.gpsimd.dma_start(
                        out=bc1, in_=dr1.to_broadcast((64, 1024)))
                    nc.gpsimd.dma_start(
                        out=bc2, in_=dr2.to_broadcast((64, 1024)))
                    u1 = attnscr.tile([64, 1024], F32, tag="u1")
                    nc.vector.tensor_mul(u1, smain[0:64, :], bc1)
                    u2 = attnscr.tile([64, 1024], F32, tag="u2")
                    nc.vector.tensor_mul(u2, smem[0:64, :], bc2)
                    nc.vector.tensor_add(attn_n[n][ho:ho + 64, :], u1, u2)

                if dbg:
                    nc.sync.dma_start(
                        out=dbg_t["dbg_q"][:, n * L:(n + 1) * L], in_=qT_n[n])
                    nc.sync.dma_start(
                        out=dbg_t["dbg_k"][:, n * L:(n + 1) * L], in_=kT_n[n])
                    nc.sync.dma_start(
                        out=dbg_t["dbg_attn"][:, n * L:(n + 1) * L],
                        in_=attn_n[n])

                # ---- out_proj partial for batch n ----
                for oc in range(8):
                    for rt2 in range(2):
                        po = pproj.tile([128, 512], F32, tag="pp")
                        nc.tensor.matmul(
                            po, wo_sb[:, oc * 128:(oc + 1) * 128],
                            attn_n[n][:, rt2 * 512:(rt2 + 1) * 512],
                            start=True, stop=True)
                        so = ostage.tile([128, 512], F32, tag="so")
                        dst = outT[oc * 128:(oc + 1) * 128,
                                   n * L + rt2 * 512:n * L + (rt2 + 1) * 512]
                        if (oc * 2 + rt2) % 4 == 0:
                            nc.scalar.activation(so, po, AF.Copy)
                            nc.scalar.dma_start(out=dst, in_=so)
                        else:
                            nc.vector.tensor_copy(so, po)
                            nc.gpsimd.dma_start(out=dst, in_=so)

    nc.compile()
    return nc


def _perm64():
    p = np.empty(64, np.int64)
    p[:32] = np.arange(0, 64, 2)
    p[32:] = np.arange(1, 64, 2)
    return p


def _prep_inputs(inputs):
    """Host-side shard prep. Returns list of per-core input dicts."""
    f = np.float32
    query = np.asarray(inputs["query"], f)
    key = np.asarray(inputs["key"], f)
    value = np.asarray(inputs["value"], f)
    W = np.asarray(inputs["in_proj_weight"], f)
    wo = np.asarray(inputs["out_proj_weight"], f)
    qp = np.asarray(inputs["qp"], f)
    kvp = np.asarray(inputs["kvp"], f)
    k_mem = np.asarray(inputs["k_mem"], f)
    v_mem = np.asarray(inputs["v_mem"], f)
    gate = np.asarray(inputs["gate_attn"], f)
    mask = np.asarray(inputs["mem_mask"]).astype(f)

    g = 1.0 / (1.0 + np.exp(-gate))
    perm64 = _perm64()
    sgn = np.concatenate([np.full(32, -1.0, f), np.full(32, 1.0, f)] * HPC)

    xqT = np.ascontiguousarray(
        query.transpose(2, 1, 0).reshape(E, R)).astype(NPBF)
    xkT = np.ascontiguousarray(
        key.transpose(2, 1, 0).reshape(E, R)).astype(NPBF)
    xvT = np.ascontiguousarray(
        value.transpose(2, 1, 0).reshape(E, R)).astype(NPBF)

    in_maps = []
    for c in range(NC):
        dims = np.arange(c * DC, (c + 1) * DC)
        dims_perm = np.concatenate([dims[h * 64 + perm64] for h in range(HPC)])
        gv = np.concatenate(
            [np.full(64, 1.0 - g[2 * c + h], f) for h in range(HPC)])

        wq = W[:E][dims_perm] * np.float32(D ** -0.5)
        wk = W[E:2 * E][dims_perm]
        wv = W[2 * E:][dims] * gv[:, None]

        def rope(pe):
            cosT = np.ascontiguousarray(
                pe[:, :, dims_perm, 0].transpose(2, 0, 1).reshape(DC, R))
            sinT = (pe[:, :, dims_perm, 1].transpose(2, 0, 1).reshape(DC, R)
                    * sgn[:, None])
            # device computes z = qraw * sin then swaps partner rows, so the
            # sin tensor itself must be pre-swapped: st[p] = sin_signed[partner(p)]
            sw = np.empty_like(sinT)
            for hb in range(HPC):
                b = hb * 64
                sw[b:b + 32] = sinT[b + 32:b + 64]
                sw[b + 32:b + 64] = sinT[b:b + 32]
            return cosT.astype(NPBF), np.ascontiguousarray(sw).astype(NPBF)

        cq, sq = rope(qp)
        ck, sk = rope(kvp)

        kmemT = np.ascontiguousarray(
            k_mem[:, dims_perm, :].transpose(1, 0, 2)).astype(NPBF)

        vma = np.zeros((N, HPC, M, 65), f)
        for n in range(N):
            for h in range(HPC):
                gh = g[2 * c + h]
                vm = v_mem[n, dims[h * 64:(h + 1) * 64], :].T  # (M, 64)
                vma[n, h, :, :64] = vm * gh * mask[n][:, None]
                vma[n, h, :, 64] = mask[n]
        vma_dev = np.ascontiguousarray(
            vma.reshape(N, HPC, 4, 128, 65).transpose(3, 0, 1, 2, 4)).astype(NPBF)

        in_maps.append({
            "xqT": xqT, "xkT": xkT, "xvT": xvT,
            "wqT": np.ascontiguousarray(wq.T).astype(NPBF),
            "wkT": np.ascontiguousarray(wk.T).astype(NPBF),
            "wvT": np.ascontiguousarray(wv.T).astype(NPBF),
            "woT": np.ascontiguousarray(wo[:, dims].T).astype(NPBF),
            "cosq": cq, "sinq": sq, "cosk": ck, "sink": sk,
            "kmem": kmemT, "vmaug": vma_dev,
        })
    return in_maps


def kernel(**inputs):
    if "nc" not in _COMPILED:
        _COMPILED["nc"] = _build()
    nc = _COMPILED["nc"]
    in_maps = _prep_inputs(inputs)
    res = bass_utils.run_bass_kernel_spmd(nc, in_maps, core_ids=list(range(NC)))
    total = np.zeros((E, R), np.float64)
    for r in res.results:
        total += r["outT"].astype(np.float64)
    out = total.T.reshape(N, L, E).transpose(1, 0, 2).astype(np.float32)
    out = out + np.asarray(inputs["out_proj_bias"], np.float32)
    return out


# revision 24
# speedup vs baseline: 1.0790x; 1.0299x over previous
"""Trainium2 Bass kernel for nn_Encoder_79585743995180 (sparse_attention).

Self-contained: hardcodes shapes/sharding. Strategy (validated in numpy):
  - 8 cores, head-parallel: core c owns heads {2c, 2c+1} (128 of 1024 dims).
  - Per core: q/k/v projections for its 128 dims (reads full activations,
    sliced weights), rope (de-interleaved even/odd permutation so the
    rotation partner sits at partition offset +32 within each 64-dim head
    block), main attention with column-softmax folded into a 1/colsum
    prescale of the AV stationary operand, memory attention with mask+gate
    folded into the host-prepped vmaug tensor, out_proj partial product.
  - Host sums the 8 partial outputs (contraction-sharded out_proj).
  - Matmul operands in bf16 (fp32 matmuls are split into hi/lo passes on
    trn2 PE = 2x instructions); accumulation stays fp32 in PSUM, and the
    softmax renormalization path stays fp32.

All biases in this problem are zeros (spec fill=zeros) and are skipped.
The reference's `+1e-8` softmax epsilon is omitted (validated: rel err
~4e-6 vs reference in fp32).

Layout conventions on device (per core):
  qT/kT   (128 dims, 4096 rows) bf16   rows r = n*L + l, dims rope-permuted
  v       rows layout, stored as v_sb (128 rows%128, 32 rowtile, 2 head, 65)
          bf16, with ones in column 64 (renorm denominator rides the AV mm)
  attnT   (128 dims, 4096 rows) bf16
  outT    (1024, 4096) fp32 partial, host sums across cores.
"""

import ml_dtypes
import numpy as np

import concourse.bass as bass
import concourse.bacc as bacc
import concourse.mybir as mybir
import concourse.tile as tile
from concourse import bass_utils

F32 = mybir.dt.float32
BF16 = mybir.dt.float16
NPBF = np.float16
AF = mybir.ActivationFunctionType

L = 1024
S = 1024
N = 4
E = 1024
H = 16
D = 64
M = 512
NC = 8
HPC = H // NC          # 2 heads per core
DC = HPC * D           # 128 dims per core
R = L * N              # 4096 rows, r = n*L + l

_COMPILED = {}


def _build(dbg=False):
    nc = bacc.Bacc("TRN2", target_bir_lowering=False, debug=False)

    # ---- DRAM I/O ----
    xqT = nc.dram_tensor("xqT", [E, R], BF16, kind="ExternalInput").ap()
    xkT = nc.dram_tensor("xkT", [E, R], BF16, kind="ExternalInput").ap()
    xvT = nc.dram_tensor("xvT", [E, R], BF16, kind="ExternalInput").ap()
    wqT = nc.dram_tensor("wqT", [E, DC], BF16, kind="ExternalInput").ap()
    wkT = nc.dram_tensor("wkT", [E, DC], BF16, kind="ExternalInput").ap()
    wvT = nc.dram_tensor("wvT", [E, DC], BF16, kind="ExternalInput").ap()
    woT = nc.dram_tensor("woT", [DC, E], BF16, kind="ExternalInput").ap()
    cosq = nc.dram_tensor("cosq", [DC, R], BF16, kind="ExternalInput").ap()
    sinq = nc.dram_tensor("sinq", [DC, R], BF16, kind="ExternalInput").ap()
    cosk = nc.dram_tensor("cosk", [DC, R], BF16, kind="ExternalInput").ap()
    sink = nc.dram_tensor("sink", [DC, R], BF16, kind="ExternalInput").ap()
    kmem = nc.dram_tensor("kmem", [DC, N, M], BF16, kind="ExternalInput").ap()
    vmaug = nc.dram_tensor("vmaug", [128, N, HPC, 4, 65], BF16,
                           kind="ExternalInput").ap()
    outT = nc.dram_tensor("outT", [E, R], BF16, kind="ExternalOutput").ap()
    dbg_t = {}
    if dbg:
        for nm, shp in (("dbg_q", [DC, R]), ("dbg_k", [DC, R]),
                        ("dbg_attn", [DC, R])):
            dbg_t[nm] = nc.dram_tensor(nm, shp, F32, kind="ExternalOutput").ap()

    with tile.TileContext(nc) as tc:
        with (
            tc.tile_pool(name="const", bufs=1) as const,
            tc.tile_pool(name="persist", bufs=1) as persist,
            tc.tile_pool(name="xstream", bufs=5) as xstream,
            tc.tile_pool(name="cs", bufs=4) as cs,
            tc.tile_pool(name="scratch", bufs=3) as scratch,
            tc.tile_pool(name="attnscr", bufs=2) as attnscr,
            tc.tile_pool(name="rows", bufs=1) as rows,
            tc.tile_pool(name="drows", bufs=2, space="DRAM") as drows,
            tc.tile_pool(name="wexp", bufs=14) as wexpp,
            tc.tile_pool(name="small", bufs=3) as small,
            tc.tile_pool(name="ostage", bufs=4) as ostage,
            tc.tile_pool(name="pw", bufs=2, space="PSUM") as pw,
            tc.tile_pool(name="pproj", bufs=2, space="PSUM") as pproj,
            tc.tile_pool(name="pacc", bufs=1, space="PSUM") as pacc,
        ):
            # ---- constants into SBUF ----
            w_sb = {}
            for name, src in (("q", wqT), ("k", wkT), ("v", wvT)):
                t = const.tile([128, 8, DC], BF16, tag=f"w_{name}")
                nc.sync.dma_start(
                    out=t, in_=src.rearrange("(kc p) d -> p kc d", p=128))
                w_sb[name] = t
            wo_sb = const.tile([DC, E], BF16)
            nc.sync.dma_start(out=wo_sb, in_=woT)
            kmem_sb = const.tile([DC, N, M], BF16)
            nc.sync.dma_start(out=kmem_sb, in_=kmem)
            vmaug_sb = const.tile([128, N, HPC, 4, 65], BF16)
            nc.sync.dma_start(out=vmaug_sb, in_=vmaug)

            # per-n persistent tiles so Tile can pipeline proj(n+1)
            # under attn/outproj(n)
            qT_n = [persist.tile([DC, L], BF16, tag=f"qT{n}", name=f"qT{n}") for n in range(N)]
            kT_n = [persist.tile([DC, L], BF16, tag=f"kT{n}", name=f"kT{n}") for n in range(N)]
            v_n = [persist.tile([128, 8, HPC, 65], BF16, tag=f"v{n}",
                                name=f"v{n}") for n in range(N)]
            attn_n = [persist.tile([DC, L], BF16, tag=f"at{n}",
                                   name=f"at{n}") for n in range(N)]
            for n in range(N):
                nc.vector.memset(v_n[n][:, :, :, 64:65], 1.0)

            for n in range(N):
                # ---- projections for batch n (rows n*L .. n*L+L) ----
                for rt2 in range(2):
                    rt = n * 2 + rt2
                    rs = slice(rt * 512, (rt + 1) * 512)
                    ls = slice(rt2 * 512, (rt2 + 1) * 512)
                    for name, xT, cosT, sinT in (
                        ("q", xqT, cosq, sinq),
                        ("k", xkT, cosk, sink),
                    ):
                        dest = qT_n[n] if name == "q" else kT_n[n]
                        xs = xstream.tile([128, 8, 512], BF16, tag="xs")
                        nc.sync.dma_start(
                            out=xs,
                            in_=xT[:, rs].rearrange("(kc p) r -> p kc r", p=128))
                        ps = pproj.tile([128, 512], F32, tag="pp")
                        for kc in range(8):
                            nc.tensor.matmul(
                                ps, w_sb[name][:, kc, :], xs[:, kc, :],
                                start=(kc == 0), stop=(kc == 7))
                        ct = cs.tile([128, 512], BF16, tag="ct")
                        st = cs.tile([128, 512], BF16, tag="st")
                        nc.sync.dma_start(out=ct, in_=cosT[:, rs])
                        nc.sync.dma_start(out=st, in_=sinT[:, rs])
                        t1 = scratch.tile([128, 512], BF16, tag="t1")
                        nc.vector.tensor_mul(t1, ps, ct)
                        z = scratch.tile([128, 512], BF16, tag="z")
                        nc.vector.tensor_mul(z, ps, st)
                        t2 = scratch.tile([128, 512], BF16, tag="t2")
                        for hb in range(HPC):
                            b = hb * 64
                            nc.gpsimd.dma_start(
                                out=t2[b:b + 32, :], in_=z[b + 32:b + 64, :])
                            nc.gpsimd.dma_start(
                                out=t2[b + 32:b + 64, :], in_=z[b:b + 32, :])
                        nc.vector.tensor_add(dest[:, ls], t1, t2)
                    # v projection for these rows
                    xs = xstream.tile([128, 8, 512], BF16, tag="xs")
                    nc.sync.dma_start(
                        out=xs,
                        in_=xvT[:, rs].rearrange("(kc p) r -> p kc r", p=128))
                    for st_i in range(4):
                        ps = pproj.tile([128, 512], F32, tag="pp")
                        for kc in range(8):
                            nc.tensor.matmul(
                                ps[:, 0:128],
                                xs[:, kc, st_i * 128:(st_i + 1) * 128],
                                w_sb["v"][:, kc, :],
                                start=(kc == 0), stop=(kc == 7))
                        t = rt2 * 4 + st_i
                        for h in range(HPC):
                            nc.scalar.activation(
                                v_n[n][:, t, h, 0:64],
                                ps[:, h * 64:(h + 1) * 64], AF.Copy)

                # ---- attention for batch n, both heads ----
                for h in range(HPC):
                    ho = h * 64
                    colsum = small.tile([128, 8], F32, tag="colsum")
                    wxs = []
                    for sc in range(8):
                        pwt = pw.tile([128, 1024], F32, tag="pw")
                        for lc in range(2):
                            nc.tensor.matmul(
                                pwt[:, lc * 512:(lc + 1) * 512],
                                kT_n[n][ho:ho + 64,
                                        sc * 128:(sc + 1) * 128],
                                qT_n[n][ho:ho + 64,
                                        lc * 512:(lc + 1) * 512],
                                start=True, stop=True)
                        wx = wexpp.tile([128, 1024], BF16, tag="wx")
                        nc.scalar.activation(
                            wx, pwt, AF.Exp, accum_out=colsum[:, sc:sc + 1])
                        wxs.append(wx)
                    rcall = small.tile([128, 8], F32, tag="rcall")
                    nc.vector.reciprocal_approx_fast(rcall, colsum)
                    pmain = pacc.tile([65, 1024], F32, tag="pmain")
                    for sc in range(8):
                        vs = small.tile([128, 65], BF16, tag="vs")
                        nc.vector.tensor_scalar_mul(
                            vs, v_n[n][:, sc, h, :], rcall[:, sc:sc + 1])
                        for lc in range(2):
                            nc.tensor.matmul(
                                pmain[:, lc * 512:(lc + 1) * 512],
                                vs, wxs[sc][:, lc * 512:(lc + 1) * 512],
                                start=(sc == 0), stop=(sc == 7))
                    pmem = pacc.tile([65, 1024], F32, tag="pmain")
                    for mc in range(4):
                        pwt = pw.tile([128, 1024], F32, tag="pw")
                        for lc in range(2):
                            nc.tensor.matmul(
                                pwt[:, lc * 512:(lc + 1) * 512],
                                kmem_sb[ho:ho + 64, n,
                                        mc * 128:(mc + 1) * 128],
                                qT_n[n][ho:ho + 64,
                                        lc * 512:(lc + 1) * 512],
                                start=True, stop=True)
                        wx = wexpp.tile([128, 1024], BF16, tag="wx")
                        nc.scalar.activation(wx, pwt, AF.Exp)
                        for lc in range(2):
                            nc.tensor.matmul(
                                pmem[:, lc * 512:(lc + 1) * 512],
                                vmaug_sb[:, n, h, mc, :],
                                wx[:, lc * 512:(lc + 1) * 512],
                                start=(mc == 0), stop=(mc == 3))
                    smain = attnscr.tile([65, 1024], F32, tag="smain")
                    smem = attnscr.tile([65, 1024], F32, tag="smem")
                    nc.scalar.activation(smain, pmain, AF.Copy)
                    nc.vector.tensor_copy(smem, pmem)
                    d1 = rows.tile([1, 1024], F32, tag="d1")
                    d2 = rows.tile([1, 1024], F32, tag="d2")
                    nc.gpsimd.dma_start(out=d1, in_=smain[64:65, :])
                    nc.gpsimd.dma_start(out=d2, in_=smem[64:65, :])
                    r1 = rows.tile([1, 1024], F32, tag="r1")
                    r2 = rows.tile([1, 1024], F32, tag="r2")
                    nc.vector.reciprocal_approx_fast(r1, d1)
                    nc.vector.reciprocal_approx_fast(r2, d2)
                    dr1 = drows.tile([1, 1024], F32, tag="dr1")
                    dr2 = drows.tile([1, 1024], F32, tag="dr2")
                    nc.gpsimd.dma_start(out=dr1, in_=r1)
                    nc.gpsimd.dma_start(out=dr2, in_=r2)
                    bc1 = attnscr.tile([64, 1024], F32, tag="bc1")
                    bc2 = attnscr.tile([64, 1024], F32, tag="bc2")
                    nc.gpsimd.dma_start(
                        out=bc1, in_=dr1.to_broadcast((64, 1024)))
                    nc.gpsimd.dma_start(
                        out=bc2, in_=dr2.to_broadcast((64, 1024)))
                    u1 = attnscr.tile([64, 1024], F32, tag="u1")
                    nc.vector.tensor_mul(u1, smain[0:64, :], bc1)
                    u2 = attnscr.tile([64, 1024], F32, tag="u2")
                    nc.vector.tensor_mul(u2, smem[0:64, :], bc2)
                    nc.vector.tensor_add(attn_n[n][ho:ho + 64, :], u1, u2)

                if dbg:
                    nc.sync.dma_start(
                        out=dbg_t["dbg_q"][:, n * L:(n + 1) * L], in_=qT_n[n])
                    nc.sync.dma_start(
                        out=dbg_t["dbg_k"][:, n * L:(n + 1) * L], in_=kT_n[n])
                    nc.sync.dma_start(
                        out=dbg_t["dbg_attn"][:, n * L:(n + 1) * L],
                        in_=attn_n[n])

                # ---- out_proj partial for batch n ----
                for oc in range(8):
                    for rt2 in range(2):
                        po = pproj.tile([128, 512], F32, tag="pp")
                        nc.tensor.matmul(
                            po, wo_sb[:, oc * 128:(oc + 1) * 128],
                            attn_n[n][:, rt2 * 512:(rt2 + 1) * 512],
                            start=True, stop=True)
                        so = ostage.tile([128, 512], BF16, tag="so")
                        dst = outT[oc * 128:(oc + 1) * 128,
                                   n * L + rt2 * 512:n * L + (rt2 + 1) * 512]
                        if (oc * 2 + rt2) % 4 == 0:
                            nc.scalar.activation(so, po, AF.Copy)
                            nc.scalar.dma_start(out=dst, in_=so)
                        else:
                            nc.vector.tensor_copy(so, po)
                            nc.gpsimd.dma_start(out=dst, in_=so)

    nc.compile()
    return nc


def _perm64():
    p = np.empty(64, np.int64)
    p[:32] = np.arange(0, 64, 2)
    p[32:] = np.arange(1, 64, 2)
    return p


def _prep_inputs(inputs):
    """Host-side shard prep. Returns list of per-core input dicts."""
    f = np.float32
    query = np.asarray(inputs["query"], f)
    key = np.asarray(inputs["key"], f)
    value = np.asarray(inputs["value"], f)
    W = np.asarray(inputs["in_proj_weight"], f)
    wo = np.asarray(inputs["out_proj_weight"], f)
    qp = np.asarray(inputs["qp"], f)
    kvp = np.asarray(inputs["kvp"], f)
    k_mem = np.asarray(inputs["k_mem"], f)
    v_mem = np.asarray(inputs["v_mem"], f)
    gate = np.asarray(inputs["gate_attn"], f)
    mask = np.asarray(inputs["mem_mask"]).astype(f)

    g = 1.0 / (1.0 + np.exp(-gate))
    perm64 = _perm64()
    sgn = np.concatenate([np.full(32, -1.0, f), np.full(32, 1.0, f)] * HPC)

    xqT = np.ascontiguousarray(
        query.transpose(2, 1, 0).reshape(E, R)).astype(NPBF)
    xkT = np.ascontiguousarray(
        key.transpose(2, 1, 0).reshape(E, R)).astype(NPBF)
    xvT = np.ascontiguousarray(
        value.transpose(2, 1, 0).reshape(E, R)).astype(NPBF)

    in_maps = []
    for c in range(NC):
        dims = np.arange(c * DC, (c + 1) * DC)
        dims_perm = np.concatenate([dims[h * 64 + perm64] for h in range(HPC)])
        gv = np.concatenate(
            [np.full(64, 1.0 - g[2 * c + h], f) for h in range(HPC)])

        wq = W[:E][dims_perm] * np.float32(D ** -0.5)
        wk = W[E:2 * E][dims_perm]
        wv = W[2 * E:][dims] * gv[:, None]

        def rope(pe):
            cosT = np.ascontiguousarray(
                pe[:, :, dims_perm, 0].transpose(2, 0, 1).reshape(DC, R))
            sinT = (pe[:, :, dims_perm, 1].transpose(2, 0, 1).reshape(DC, R)
                    * sgn[:, None])
            # device computes z = qraw * sin then swaps partner rows, so the
            # sin tensor itself must be pre-swapped: st[p] = sin_signed[partner(p)]
            sw = np.empty_like(sinT)
            for hb in range(HPC):
                b = hb * 64
                sw[b:b + 32] = sinT[b + 32:b + 64]
                sw[b + 32:b + 64] = sinT[b:b + 32]
            return cosT.astype(NPBF), np.ascontiguousarray(sw).astype(NPBF)

        cq, sq = rope(qp)
        ck, sk = rope(kvp)

        kmemT = np.ascontiguousarray(
            k_mem[:, dims_perm, :].transpose(1, 0, 2)).astype(NPBF)

        vma = np.zeros((N, HPC, M, 65), f)
        for n in range(N):
            for h in range(HPC):
                gh = g[2 * c + h]
                vm = v_mem[n, dims[h * 64:(h + 1) * 64], :].T  # (M, 64)
                vma[n, h, :, :64] = vm * gh * mask[n][:, None]
                vma[n, h, :, 64] = mask[n]
        vma_dev = np.ascontiguousarray(
            vma.reshape(N, HPC, 4, 128, 65).transpose(3, 0, 1, 2, 4)).astype(NPBF)

        in_maps.append({
            "xqT": xqT, "xkT": xkT, "xvT": xvT,
            "wqT": np.ascontiguousarray(wq.T).astype(NPBF),
            "wkT": np.ascontiguousarray(wk.T).astype(NPBF),
            "wvT": np.ascontiguousarray(wv.T).astype(NPBF),
            "woT": np.ascontiguousarray(wo[:, dims].T).astype(NPBF),
            "cosq": cq, "sinq": sq, "cosk": ck, "sink": sk,
            "kmem": kmemT, "vmaug": vma_dev,
        })
    return in_maps


def kernel(**inputs):
    if "nc" not in _COMPILED:
        _COMPILED["nc"] = _build()
    nc = _COMPILED["nc"]
    in_maps = _prep_inputs(inputs)
    res = bass_utils.run_bass_kernel_spmd(nc, in_maps, core_ids=list(range(NC)))
    total = np.zeros((E, R), np.float64)
    for r in res.results:
        total += r["outT"].astype(np.float64)
    out = total.T.reshape(N, L, E).transpose(1, 0, 2).astype(np.float32)
    out = out + np.asarray(inputs["out_proj_bias"], np.float32)
    return out
